# revision 36
# baseline (speedup 1.0000x reference)
"""Multi-head attention (B=1, S=4096, dim=1024, H=16, hd=64) on 8 TRN2 cores.

Sharding: tensor-parallel over heads — 2 heads per core. Wq/Wk/Wv are
column-split (each core computes its 128 output dims of Q/K/V), Wo row-split
(each core computes the full-depth contraction for its 128 output columns
after an AllGather of the per-core attention outputs).

Layout strategy: everything on device is transposed so that every matmul
contraction lands on the partition axis with zero DMA transposes:
  - host passes x.T, pre-tiled Wq.T/Wk.T/Wv.T/Wo.T slices (all bf16)
  - Q.T, K.T, V.T computed as [e, s] (e on partitions); V then PE-transposed
    per 128x128 tile into [s, e]
  - scores computed transposed: S_T[k, q]; the two heads occupy disjoint
    PE row groups (rows 0-63 / 64-127) so their matmuls run concurrently
  - softmax: exp on ScalarE (scale=1/8 folded in, no max subtraction —
    scores are N(0, ~0.41^2), |s|max ~4); ScalarE is the bottleneck engine
    (1 elem/cycle/lane), so everything else is scheduled under its ~1us
    per-(qc,kt) exp cadence
  - AV: the two heads run as concurrent PE column-tiles (h0 -> output
    partitions 0-63, h1 -> 64-127), accumulating a [128, 512] f32 PSUM
    tile per q-chunk (double-buffered)
  - softmax denominators: the otherwise-idle DVE accumulates
    esum[p,q] += exp-tile (bf16) per k-tile; once per q-chunk a tiny
    M=1 ones-matmul reduces esum over partitions, a [128,8]-reshaped
    reciprocal (via 2 tiny DMAs) inverts it, and two K=1 broadcast
    matmuls expand 1/denom to all 128 partitions; the normalize multiply
    reads the AV accumulator straight out of PSUM
  - the ramp: x is DMA'd in 512-column chunks with the K projection
    starting on chunk 0 immediately; remaining K/V/Q projections and V
    transposes are emitted as deadline-ordered fine-grained filler
    generators (t-major pairs amortize LDWEIGHTS) that ping-pong between
    the spare PSUM slot and the (initially free) AV accumulator slots
  - AllGather of normalized attn.T (bf16), one 512-column chunk per
    q-chunk, fired the moment its normalize completes; bounce + norm
    reshape DMAs ride the gpsimd queue, gathered-chunk loads own the
    sync queue (a queued DMA blocks its queue until its input is ready,
    so AG-gated loads must not share a queue with anything earlier)
  - output projection per gathered chunk overlaps the remaining AGs;
    out.T [128 e_out, 4096 s] is transposed on the host.
"""

import numpy as np
import ml_dtypes

N_CORES = 8
S = 4096
DIM = 1024
HD = 64
EC = 128          # output dims (= 2 heads * 64) per core
QC = 512          # q-chunk width in the main loop
NQC = S // QC
KT = S // 128     # 32 k-tiles
DT = DIM // 128   # 8 d-tiles
NAG = 8           # AllGather chunks along s
SAG = S // NAG
LAG = 24          # AV software-pipeline depth (k-tiles behind scores)

_cached = {}


def _build(debug=False):
    import concourse.bass as bass
    import concourse.mybir as mybir
    import concourse.tile as tile
    from concourse import bacc
    from concourse.masks import make_identity

    BF = mybir.dt.bfloat16
    F32 = mybir.dt.float32
    MULT = mybir.AluOpType.mult
    EXP = mybir.ActivationFunctionType.Exp

    nc = bacc.Bacc("TRN2", target_bir_lowering=False, debug=False,
                   num_devices=N_CORES)

    xt_d = nc.declare_dram_parameter("xt", [DIM, S], BF, isOutput=False)
    wqt_d = nc.declare_dram_parameter("wqt", [128, DT * EC], BF, isOutput=False)
    wkt_d = nc.declare_dram_parameter("wkt", [128, DT * EC], BF, isOutput=False)
    wvt_d = nc.declare_dram_parameter("wvt", [128, DT * EC], BF, isOutput=False)
    wot_d = nc.declare_dram_parameter("wot", [128, DT * EC], BF, isOutput=False)
    out_d = nc.declare_dram_parameter("out_t", [EC, S], F32, isOutput=True)

    AGW = [SAG] * NAG
    bounce = [nc.dram_tensor(f"bounce{j}", [EC, w], BF)
              for j, w in enumerate(AGW)]
    ag_out = [nc.dram_tensor(f"ag_out{j}", [DIM, w], BF, addr_space="Shared")
              for j, w in enumerate(AGW)]

    if debug:
        dbg_denom = nc.declare_dram_parameter("dbg_denom", [1, 512], F32,
                                              isOutput=True)
        dbg_rcp = nc.declare_dram_parameter("dbg_rcp", [1, 512], F32,
                                            isOutput=True)
        dbg_rb = nc.declare_dram_parameter("dbg_rb", [64, 512], F32,
                                           isOutput=True)
        dbg_pt = nc.declare_dram_parameter("dbg_pt", [128, 1024], BF,
                                           isOutput=True)

    with tile.TileContext(nc) as tc:
        with (
            tc.tile_pool(name="const", bufs=1) as cpool,
            tc.tile_pool(name="pt", bufs=LAG + 4) as ptp,
            tc.tile_pool(name="norm", bufs=2) as npool,
            tc.tile_pool(name="esum", bufs=2) as esp,
            tc.tile_pool(name="ps_sc", bufs=2, space="PSUM") as psc,
            tc.tile_pool(name="ps_fl", bufs=1, space="PSUM") as ps2,
            tc.tile_pool(name="ps_acc", bufs=2, space="PSUM") as pac,
        ):
            # ---- persistent SBUF tiles ----
            wq_sb = cpool.tile([128, DT, EC], BF, tag="wq")
            wk_sb = cpool.tile([128, DT, EC], BF, tag="wk")
            wv_sb = cpool.tile([128, DT, EC], BF, tag="wv")
            wo_sb = cpool.tile([128, DT, EC], BF, tag="wo")
            xt_sb = cpool.tile([128, DT, S], BF, tag="big")
            qt_sb = cpool.tile([128, S], BF, tag="qt")
            kt_sb = cpool.tile([128, S], BF, tag="kt")
            vt_sb = cpool.tile([128, S], BF, tag="vt")
            v_sb = cpool.tile([128, KT, 128], BF, tag="v")
            attnt_sb = cpool.tile([128, S], BF, tag="attnt")
            ident = cpool.tile([128, 128], BF, tag="ident")
            ones_sb = cpool.tile([128, 64], BF, tag="ones")
            outsb = cpool.tile([128, S], F32, tag="outsb")

            # ---- loads: K weights + x chunk 0 first so the K projection
            # starts immediately; remaining x chunks stream in behind ----
            nc.sync.dma_start(
                wk_sb[:], wkt_d.rearrange("p (o f) -> p o f", o=DT))

            # one trigger per 512-column chunk (3D AP over the 8 row
            # blocks) — the sync sequencer dispatches triggers serially at
            # ~150ns each, so 64 separate transfers would delay the first
            # data (and the whole exp stream) by ~10us
            xt_dv = xt_d.rearrange("(t p) s -> p t s", p=128)

            def load_chunk(j):
                js = slice(j * QC, (j + 1) * QC)
                nc.sync.dma_start(xt_sb[:, :, js], xt_dv[:, :, js])

            load_chunk(0)
            for wsb, wd in ((wq_sb, wqt_d), (wv_sb, wvt_d), (wo_sb, wot_d)):
                nc.sync.dma_start(
                    wsb[:], wd.rearrange("p (o f) -> p o f", o=DT))
            for j in range(1, DT):
                load_chunk(j)
            make_identity(nc, ident[:])
            nc.vector.memset(ones_sb[:], 1.0)

            # load the exp table set (~2.7us) during the x DMA
            wrm = ps2.tile([64, 64], F32, tag="s2", name="warm")
            nc.scalar.activation(wrm[0:1, 0:16], ident[0:1, 0:16], EXP)

            # ---- emission helpers ----
            pts = {}
            esums = {}
            rbs = {}
            rcpbs = {}

            def emit_scores(qc, kt):
                qs = slice(qc * QC, (qc + 1) * QC)
                sc = psc.tile([128, 1024], F32, tag="sc")
                for h in (0, 1):
                    nc.tensor.matmul(
                        sc[:, h * 512:(h + 1) * 512],
                        lhsT=kt_sb[h * 64:(h + 1) * 64,
                                   kt * 128:(kt + 1) * 128],
                        rhs=qt_sb[h * 64:(h + 1) * 64, qs],
                        start=True, stop=True)
                pt = ptp.tile([128, 1024], BF, tag="pt")
                nc.scalar.activation(pt[:], sc[:], EXP, scale=0.125)
                pts[(qc, kt)] = pt
                if debug and qc == 0 and kt == 0:
                    nc.sync.dma_start(dbg_pt[:, :], pt[:])

            def emit_esum(qc, kt):
                # running softmax-denominator accumulation on the (idle)
                # DVE: esum[p, q] = sum_kt pt[p, q]; the partition-axis
                # reduction happens later in one tiny ones-matmul per qc.
                # bf16 accumulation noise on the final denominator is
                # ~0.1% (128 independent partials averaged by the matmul).
                pt = pts[(qc, kt)]
                if kt == 0:
                    es = esp.tile([128, 1024], BF, tag="es", name=f"es{qc}")
                    esums[qc] = es
                    nc.vector.tensor_copy(out=es[:], in_=pt[:])
                else:
                    es = esums[qc]
                    nc.vector.tensor_tensor(es[:], es[:], pt[:],
                                            mybir.AluOpType.add)

            def emit_dd(qc):
                # partition-reduce esum -> per-q denominators [1, 1024],
                # then 128-lane reciprocal via tiny reshape DMAs
                es = esums.pop(qc)
                dd = ps2.tile([1, 1024], F32, tag="s2", name=f"dd{qc}")
                for h in (0, 1):
                    nc.tensor.matmul(
                        dd[0:1, h * 512:(h + 1) * 512],
                        lhsT=ones_sb[:, 0:1],
                        rhs=es[:, h * 512:(h + 1) * 512],
                        start=True, stop=True)
                dd_sb = npool.tile([1, 1024], F32, tag="dd")
                nc.vector.tensor_copy(out=dd_sb[0:1, :], in_=dd[0:1, :])
                den = npool.tile([128, 8], F32, tag="den")
                nc.gpsimd.dma_start(den[:, :], dd_sb[0:1, :])
                rcp = npool.tile([128, 8], F32, tag="rcp")
                nc.vector.reciprocal(rcp[:, :], den[:, :])
                rcpb4 = npool.tile([128, 8], BF, tag="rcpb4")
                nc.vector.tensor_copy(out=rcpb4[:], in_=rcp[:])
                rcpb = npool.tile([1, 1024], BF, tag="rcpb")
                nc.gpsimd.dma_start(rcpb[0:1, :], rcpb4[:, :])
                rcpbs[qc] = rcpb

            def emit_bc(qc):
                # broadcast 1/denom rows to all 128 partitions (h0 rows on
                # partitions 0-63, h1 on 64-127) via two K=1 matmuls
                rcpb = rcpbs.pop(qc)
                bcps = ps2.tile([128, 512], F32, tag="s2", name=f"bc{qc}")
                for h in (0, 1):
                    nc.tensor.matmul(
                        bcps[h * 64:(h + 1) * 64, :],
                        lhsT=ones_sb[0:1, 0:64],
                        rhs=rcpb[0:1, h * 512:(h + 1) * 512],
                        start=True, stop=True)
                rb = npool.tile([128, 512], F32, tag="rb")
                nc.vector.tensor_copy(out=rb[:], in_=bcps[:])
                if debug and qc == 0:
                    nc.sync.dma_start(dbg_rb[:, :], rb[0:64, :])
                rbs[qc] = rb

            def emit_av(qc, kt, acc):
                # the two heads run as concurrent PE column-tiles:
                # h0 -> output partitions 0-63, h1 -> 64-127
                pt = pts.pop((qc, kt))
                for h in (0, 1):
                    nc.tensor.matmul(
                        acc[h * 64:(h + 1) * 64, :],
                        lhsT=v_sb[:, kt, h * 64:(h + 1) * 64],
                        rhs=pt[:, h * 512:(h + 1) * 512],
                        start=(kt == 0), stop=(kt == KT - 1))

            def fire_ag(j, js):
                nc.gpsimd.dma_start(bounce[j][:, :], attnt_sb[:, js])
                nc.gpsimd.collective_compute(
                    "AllGather",
                    mybir.AluOpType.bypass,
                    replica_groups=[list(range(N_CORES))],
                    ins=[bounce[j].ap().opt()],
                    outs=[ag_out[j].ap().opt()],
                )

            def emit_norm_and_ag(qc, acc):
                # normalize straight out of PSUM (rb is long ready), ship
                qs = slice(qc * QC, (qc + 1) * QC)
                nc.vector.tensor_tensor(
                    attnt_sb[:, qs], acc[:, :], rbs.pop(qc)[:, :], MULT)
                fire_ag(qc, qs)

            # ---- stage 1: chunked projections ----
            _pc = [0]

            def proj_gen(wsb, dest, j, pool=None, ptag=None):
                # one chunk of one projection; single 1-bank PSUM buf,
                # yields every 2 t-steps so scores/exp interleave finely
                _pc[0] += 1
                pool = pool or ps2
                ps = pool.tile([128, 512], F32, tag=ptag or "s2",
                               name=f"pj{_pc[0]}")
                js = slice(j * 512, (j + 1) * 512)
                for t in range(DT):
                    nc.tensor.matmul(
                        ps[:],
                        lhsT=wsb[:, t, :],
                        rhs=xt_sb[:, t, js],
                        start=(t == 0), stop=(t == DT - 1))
                    if t % 2 == 1 and t < DT - 1:
                        yield
                nc.vector.tensor_copy(out=dest[:, js], in_=ps[:])
                yield

            def proj_pair_gen(wsb, dest, j0, pool=None, ptag=None):
                # two 512-col chunks t-major in one [128,1024] PSUM buf:
                # the per-t LDWEIGHTS is amortized over both matmuls
                _pc[0] += 1
                pool = pool or ps2
                ps = pool.tile([128, 1024], F32, tag=ptag or "s2",
                               name=f"pp{_pc[0]}")
                for t in range(DT):
                    for jj in (0, 1):
                        nc.tensor.matmul(
                            ps[:, jj * 512:(jj + 1) * 512],
                            lhsT=wsb[:, t, :],
                            rhs=xt_sb[:, t,
                                      (j0 + jj) * 512:(j0 + jj + 1) * 512],
                            start=(t == 0), stop=(t == DT - 1))
                    yield
                nc.vector.tensor_copy(
                    out=dest[:, j0 * 512:(j0 + 2) * 512], in_=ps[:])
                yield

            def transpose_gen(jv, pool=None, ptag=None):
                # 4 PE transposes of V into one PSUM tile + one grouped copy
                pool = pool or ps2
                tp = pool.tile([128, 4, 128], BF, tag=ptag or "s2",
                               name=f"tp{jv}")
                for i in range(4):
                    st = 4 * jv + i
                    nc.tensor.transpose(
                        tp[:, i, :], vt_sb[:, st * 128:(st + 1) * 128],
                        ident[:])
                    if i == 1:
                        yield
                nc.vector.tensor_copy(
                    out=v_sb[:, 4 * jv:4 * jv + 4, :], in_=tp[:])
                yield

            def run_gen(g):
                for _ in g:
                    pass

            # prologue: K chunk 0 and Q chunk 0 gate the first scores
            # (different PSUM pools, so Q0's matmuls start right behind
            # K0's without waiting for K0's eviction)
            run_gen(proj_gen(wk_sb, kt_sb, 0))
            run_gen(proj_gen(wq_sb, qt_sb, 0, pac, "acc"))

            # deadline-ordered fillers (consumer emission positions):
            # K_j needed by scores step 4j; V_jv+T_jv by AV step 12+4jv;
            # Q_j by step 32j. Emitted at ~3 yields/step.
            from collections import deque
            fillers = deque()
            fillers.append(proj_pair_gen(wk_sb, kt_sb, 1))
            fillers.append(proj_gen(wv_sb, vt_sb, 0, pac, "acc"))
            fillers.append(proj_gen(wv_sb, vt_sb, 1, pac, "acc"))
            fillers.append(transpose_gen(0, pac, "acc"))
            fillers.append(transpose_gen(1, pac, "acc"))
            fillers.append(proj_pair_gen(wk_sb, kt_sb, 3))
            fillers.append(proj_gen(wv_sb, vt_sb, 2, pac, "acc"))
            fillers.append(proj_gen(wv_sb, vt_sb, 3, pac, "acc"))
            fillers.append(transpose_gen(2, pac, "acc"))
            fillers.append(transpose_gen(3, pac, "acc"))
            fillers.append(proj_pair_gen(wk_sb, kt_sb, 5))
            fillers.append(proj_gen(wv_sb, vt_sb, 4, pac, "acc"))
            fillers.append(transpose_gen(4, pac, "acc"))
            fillers.append(proj_gen(wk_sb, kt_sb, 7, pac, "acc"))
            fillers.append(proj_gen(wv_sb, vt_sb, 5, pac, "acc"))
            fillers.append(transpose_gen(5, pac, "acc"))
            fillers.append(proj_gen(wq_sb, qt_sb, 1, pac, "acc"))
            fillers.append(proj_pair_gen(wv_sb, vt_sb, 6))
            fillers.append(transpose_gen(6, pac, "acc"))
            fillers.append(transpose_gen(7, pac, "acc"))
            fillers.append(proj_pair_gen(wq_sb, qt_sb, 2))
            fillers.append(proj_gen(wq_sb, qt_sb, 4, pac, "acc"))
            fillers.append(proj_gen(wq_sb, qt_sb, 5, pac, "acc"))
            fillers.append(proj_pair_gen(wq_sb, qt_sb, 6))

            def filler_step():
                while fillers:
                    try:
                        next(fillers[0])
                        return
                    except StopIteration:
                        fillers.popleft()

            ag_sb = cpool.tile([128, DT, S], BF, tag="big")

            # ---- stage 2: flat software-pipelined attention loop ----
            ESLAG = 8
            seq = [(qc, kt) for qc in range(NQC) for kt in range(KT)]
            accs = {}
            dd_at = {}
            bc_at = {}

            def do_av(g):
                qc, kt = seq[g]
                if kt == 0:
                    accs[qc] = pac.tile([128, 512], F32, tag="acc",
                                        name=f"acc{qc}")
                emit_av(qc, kt, accs[qc])
                if kt == KT - 1:
                    emit_norm_and_ag(qc, accs.pop(qc))
                    nc.sync.dma_start(
                        ag_sb[:, :, qc * QC:(qc + 1) * QC],
                        ag_out[qc].rearrange("(t p) s -> p t s", p=128))

            av_at = {}
            for g, (qc, kt) in enumerate(seq):
                lag = (LAG if qc < NQC - 2 else
                       (20 if qc == NQC - 2 else 16))
                av_at.setdefault(g + lag, []).append(g)
            for qc in range(NQC):
                g31 = qc * KT + KT - 1
                if qc < NQC - 1:
                    dd_at[g31 + ESLAG + 8] = qc
                    bc_at[g31 + ESLAG + 12] = qc
                else:
                    dd_at[g31 + ESLAG + 1] = qc
                    bc_at[g31 + ESLAG + 3] = qc

            for g in range(len(seq) + LAG):
                if g < len(seq):
                    emit_scores(*seq[g])
                filler_step()
                filler_step()
                if g < 21:
                    filler_step()
                if ESLAG <= g < len(seq) + ESLAG:
                    emit_esum(*seq[g - ESLAG])
                if g in dd_at:
                    emit_dd(dd_at.pop(g))
                if g in bc_at:
                    emit_bc(bc_at.pop(g))
                for gg in av_at.pop(g, ()):
                    do_av(gg)

            # ---- stage 4: output projection (ag_sb chunks already
            # loaded as each AllGather completed) ----
            for j in range(S // 512):
                pool, tag = ((ps2, "s2") if j % 2 == 0 else (pac, "acc"))
                ps = pool.tile([128, 512], F32, tag=tag, name=f"po{j}")
                for t in range(DT):
                    nc.tensor.matmul(
                        ps[:],
                        lhsT=wo_sb[:, t, :],
                        rhs=ag_sb[:, t, j * 512:(j + 1) * 512],
                        start=(t == 0), stop=(t == DT - 1))
                nc.vector.tensor_copy(
                    out=outsb[:, j * 512:(j + 1) * 512], in_=ps[:])
                nc.sync.dma_start(out_d[:, j * 512:(j + 1) * 512],
                                  outsb[:, j * 512:(j + 1) * 512])

    nc.finalize()
    return nc


def _get_nc():
    if "nc" not in _cached:
        _cached["nc"] = _build()
    return _cached["nc"]


def _tile_w(wslice):
    # [1024, 128] -> [128, DT*128] partition-major tiling (bf16, contiguous)
    bf16 = ml_dtypes.bfloat16
    return np.ascontiguousarray(
        wslice.reshape(DT, 128, EC).transpose(1, 0, 2).reshape(128, DT * EC)
    ).astype(bf16)


def _prep_inputs(x, Wq, Wk, Wv, Wo):
    bf16 = ml_dtypes.bfloat16
    x2d = np.asarray(x, dtype=np.float32).reshape(S, DIM)
    xt = np.ascontiguousarray(x2d.T).astype(bf16)
    Wq = np.asarray(Wq, dtype=np.float32)
    Wk = np.asarray(Wk, dtype=np.float32)
    Wv = np.asarray(Wv, dtype=np.float32)
    Wo = np.asarray(Wo, dtype=np.float32)
    in_maps = []
    for c in range(N_CORES):
        sl = slice(c * EC, (c + 1) * EC)
        in_maps.append({
            "xt": xt,
            "wqt": _tile_w(Wq[sl].T),
            "wkt": _tile_w(Wk[sl].T),
            "wvt": _tile_w(Wv[sl].T),
            "wot": _tile_w(Wo[sl].T),
        })
    return in_maps


def run(x, Wq, Wk, Wv, Wo, trace=False):
    """Run the SPMD kernel; returns (out [1,S,DIM] f32, BassKernelResults)."""
    from concourse.bass_utils import run_bass_kernel_spmd

    if trace:
        try:
            import profhook
            profhook.install()
        except Exception:
            pass
    nc = _get_nc()
    in_maps = _prep_inputs(x, Wq, Wk, Wv, Wo)
    res = run_bass_kernel_spmd(nc, in_maps, core_ids=list(range(N_CORES)),
                               trace=trace)
    out = np.empty((1, S, DIM), dtype=np.float32)
    for c in range(N_CORES):
        out[0, :, c * EC:(c + 1) * EC] = res.results[c]["out_t"].T
    return out, res


def kernel(x, mask, Wq, Wk, Wv, Wo):
    # mask is all-zeros by problem spec; it is not applied on device.
    out, _ = run(x, Wq, Wk, Wv, Wo, trace=False)
    return out


# revision 37
# speedup vs baseline: 1.1050x; 1.1050x over previous
"""Multi-head attention (B=1, S=4096, dim=1024, H=16, hd=64) on 8 TRN2 cores.

Sharding: tensor-parallel over heads — 2 heads per core. Wq/Wk/Wv are
column-split (each core computes its 128 output dims of Q/K/V), Wo row-split
(each core computes the full-depth contraction for its 128 output columns
after an AllGather of the per-core attention outputs).

Layout strategy: everything on device is transposed so that every matmul
contraction lands on the partition axis with zero DMA transposes:
  - host passes x.T, pre-tiled Wq.T/Wk.T/Wv.T/Wo.T slices (all bf16)
  - Q.T, K.T, V.T computed as [e, s] (e on partitions); V then PE-transposed
    per 128x128 tile into [s, e]
  - scores computed transposed: S_T[k, q]; the two heads occupy disjoint
    PE row groups (rows 0-63 / 64-127) so their matmuls run concurrently
  - softmax: exp on ScalarE (scale=1/8 folded in, no max subtraction —
    scores are N(0, ~0.41^2), |s|max ~4); ScalarE is the bottleneck engine
    (1 elem/cycle/lane), so everything else is scheduled under its ~1us
    per-(qc,kt) exp cadence
  - AV: the two heads run as concurrent PE column-tiles (h0 -> output
    partitions 0-63, h1 -> 64-127), accumulating a [128, 512] f32 PSUM
    tile per q-chunk (double-buffered)
  - softmax denominators: the otherwise-idle DVE accumulates
    esum[p,q] += exp-tile (bf16) per k-tile; once per q-chunk a tiny
    M=1 ones-matmul reduces esum over partitions, a [128,8]-reshaped
    reciprocal (via 2 tiny DMAs) inverts it, and two K=1 broadcast
    matmuls expand 1/denom to all 128 partitions; the normalize multiply
    reads the AV accumulator straight out of PSUM
  - the ramp: x is DMA'd in 512-column chunks with the K projection
    starting on chunk 0 immediately; remaining K/V/Q projections and V
    transposes are emitted as deadline-ordered fine-grained filler
    generators (t-major pairs amortize LDWEIGHTS) that ping-pong between
    the spare PSUM slot and the (initially free) AV accumulator slots
  - AllGather of normalized attn.T (bf16), one 512-column chunk per
    q-chunk, fired the moment its normalize completes; bounce + norm
    reshape DMAs ride the gpsimd queue, gathered-chunk loads own the
    sync queue (a queued DMA blocks its queue until its input is ready,
    so AG-gated loads must not share a queue with anything earlier)
  - output projection per gathered chunk overlaps the remaining AGs;
    out.T [128 e_out, 4096 s] is transposed on the host.
"""

import numpy as np
import ml_dtypes

N_CORES = 8
S = 4096
DIM = 1024
HD = 64
EC = 128          # output dims (= 2 heads * 64) per core
QC = 512          # q-chunk width in the main loop
NQC = S // QC
KT = S // 128     # 32 k-tiles
DT = DIM // 128   # 8 d-tiles
NAG = 8           # AllGather chunks along s
SAG = S // NAG
LAG = 24          # AV software-pipeline depth (k-tiles behind scores)

_cached = {}


def _build(debug=False):
    import concourse.bass as bass
    import concourse.mybir as mybir
    import concourse.tile as tile
    from concourse import bacc
    from concourse.masks import make_identity

    BF = mybir.dt.bfloat16
    F32 = mybir.dt.float32
    MULT = mybir.AluOpType.mult
    EXP = mybir.ActivationFunctionType.Exp

    nc = bacc.Bacc("TRN2", target_bir_lowering=False, debug=False,
                   num_devices=N_CORES)

    xt_d = nc.declare_dram_parameter("xt", [DIM, S], BF, isOutput=False)
    wqt_d = nc.declare_dram_parameter("wqt", [128, DT * EC], BF, isOutput=False)
    wkt_d = nc.declare_dram_parameter("wkt", [128, DT * EC], BF, isOutput=False)
    wvt_d = nc.declare_dram_parameter("wvt", [128, DT * EC], BF, isOutput=False)
    wot_d = nc.declare_dram_parameter("wot", [128, DT * EC], BF, isOutput=False)
    out_d = nc.declare_dram_parameter("out_t", [EC, S], F32, isOutput=True)

    AGW = [SAG] * NAG
    bounce = [nc.dram_tensor(f"bounce{j}", [EC, w], BF)
              for j, w in enumerate(AGW)]
    ag_out = [nc.dram_tensor(f"ag_out{j}", [DIM, w], BF, addr_space="Shared")
              for j, w in enumerate(AGW)]

    if debug:
        dbg_denom = nc.declare_dram_parameter("dbg_denom", [1, 512], F32,
                                              isOutput=True)
        dbg_rcp = nc.declare_dram_parameter("dbg_rcp", [1, 512], F32,
                                            isOutput=True)
        dbg_rb = nc.declare_dram_parameter("dbg_rb", [64, 512], F32,
                                           isOutput=True)
        dbg_pt = nc.declare_dram_parameter("dbg_pt", [128, 1024], BF,
                                           isOutput=True)

    with tile.TileContext(nc) as tc:
        with (
            tc.tile_pool(name="const", bufs=1) as cpool,
            tc.tile_pool(name="pt", bufs=LAG + 4) as ptp,
            tc.tile_pool(name="norm", bufs=2) as npool,
            tc.tile_pool(name="esum", bufs=2) as esp,
            tc.tile_pool(name="ps_sc", bufs=2, space="PSUM") as psc,
            tc.tile_pool(name="ps_fl", bufs=1, space="PSUM") as ps2,
            tc.tile_pool(name="ps_acc", bufs=2, space="PSUM") as pac,
        ):
            # ---- persistent SBUF tiles ----
            wq_sb = cpool.tile([128, DT, EC], BF, tag="wq")
            wk_sb = cpool.tile([128, DT, EC], BF, tag="wk")
            wv_sb = cpool.tile([128, DT, EC], BF, tag="wv")
            wo_sb = cpool.tile([128, DT, EC], BF, tag="wo")
            xt_sb = cpool.tile([128, DT, S], BF, tag="big")
            qt_sb = cpool.tile([128, S], BF, tag="qt")
            kt_sb = cpool.tile([128, S], BF, tag="kt")
            vt_sb = cpool.tile([128, S], BF, tag="vt")
            v_sb = cpool.tile([128, KT, 128], BF, tag="v")
            attnt_sb = cpool.tile([128, S], BF, tag="attnt")
            ident = cpool.tile([128, 128], BF, tag="ident")
            ones_sb = cpool.tile([128, 64], BF, tag="ones")
            outsb = cpool.tile([128, S], F32, tag="outsb")

            # ---- loads: K weights + x chunk 0 first so the K projection
            # starts immediately; remaining x chunks stream in behind ----
            nc.sync.dma_start(
                wk_sb[:], wkt_d.rearrange("p (o f) -> p o f", o=DT))

            # one trigger per 512-column chunk (3D AP over the 8 row
            # blocks) — the sync sequencer dispatches triggers serially at
            # ~150ns each, so 64 separate transfers would delay the first
            # data (and the whole exp stream) by ~10us
            xt_dv = xt_d.rearrange("(t p) s -> p t s", p=128)

            def load_chunk(j):
                js = slice(j * QC, (j + 1) * QC)
                nc.sync.dma_start(xt_sb[:, :, js], xt_dv[:, :, js])

            load_chunk(0)
            for wsb, wd in ((wq_sb, wqt_d), (wv_sb, wvt_d), (wo_sb, wot_d)):
                nc.sync.dma_start(
                    wsb[:], wd.rearrange("p (o f) -> p o f", o=DT))
            for j in range(1, DT):
                load_chunk(j)
            make_identity(nc, ident[:])
            nc.vector.memset(ones_sb[:], 1.0)

            # load the exp table set (~2.7us) during the x DMA, and keep
            # the PE busy ~3.5us right before the K projection so HAM has
            # unthrottled to full clock when real work starts (gated on
            # ident so it runs at ~10.5us, ending as chunk-0 data lands)
            wrm = ps2.tile([64, 64], F32, tag="s2", name="warm")
            nc.scalar.activation(wrm[0:1, 0:16], ident[0:1, 0:16], EXP)
            for _ in range(40):
                nc.tensor.matmul(wrm[:], lhsT=ident[0:64, 0:64],
                                 rhs=ident[0:64, 64:128], start=True,
                                 stop=True)

            # ---- emission helpers ----
            pts = {}
            esums = {}
            rbs = {}
            rcpbs = {}

            def emit_scores(qc, kt):
                qs = slice(qc * QC, (qc + 1) * QC)
                sc = psc.tile([128, 1024], F32, tag="sc")
                for h in (0, 1):
                    nc.tensor.matmul(
                        sc[:, h * 512:(h + 1) * 512],
                        lhsT=kt_sb[h * 64:(h + 1) * 64,
                                   kt * 128:(kt + 1) * 128],
                        rhs=qt_sb[h * 64:(h + 1) * 64, qs],
                        start=True, stop=True)
                pt = ptp.tile([128, 1024], BF, tag="pt")
                nc.scalar.activation(pt[:], sc[:], EXP, scale=0.125)
                pts[(qc, kt)] = pt
                if debug and qc == 0 and kt == 0:
                    nc.sync.dma_start(dbg_pt[:, :], pt[:])

            def emit_esum(qc, kt):
                # running softmax-denominator accumulation on the (idle)
                # DVE: esum[p, q] = sum_kt pt[p, q]; the partition-axis
                # reduction happens later in one tiny ones-matmul per qc.
                # bf16 accumulation noise on the final denominator is
                # ~0.1% (128 independent partials averaged by the matmul).
                pt = pts[(qc, kt)]
                if kt == 0:
                    es = esp.tile([128, 1024], BF, tag="es", name=f"es{qc}")
                    esums[qc] = es
                    nc.vector.tensor_copy(out=es[:], in_=pt[:])
                else:
                    es = esums[qc]
                    nc.vector.tensor_tensor(es[:], es[:], pt[:],
                                            mybir.AluOpType.add)

            def emit_dd(qc):
                # partition-reduce esum -> per-q denominators [1, 1024],
                # then 128-lane reciprocal via tiny reshape DMAs
                es = esums.pop(qc)
                dd = ps2.tile([1, 1024], F32, tag="s2", name=f"dd{qc}")
                for h in (0, 1):
                    nc.tensor.matmul(
                        dd[0:1, h * 512:(h + 1) * 512],
                        lhsT=ones_sb[:, 0:1],
                        rhs=es[:, h * 512:(h + 1) * 512],
                        start=True, stop=True)
                dd_sb = npool.tile([1, 1024], F32, tag="dd")
                nc.vector.tensor_copy(out=dd_sb[0:1, :], in_=dd[0:1, :])
                den = npool.tile([128, 8], F32, tag="den")
                nc.gpsimd.dma_start(den[:, :], dd_sb[0:1, :])
                rcp = npool.tile([128, 8], F32, tag="rcp")
                nc.vector.reciprocal(rcp[:, :], den[:, :])
                rcpb4 = npool.tile([128, 8], BF, tag="rcpb4")
                nc.vector.tensor_copy(out=rcpb4[:], in_=rcp[:])
                rcpb = npool.tile([1, 1024], BF, tag="rcpb")
                nc.gpsimd.dma_start(rcpb[0:1, :], rcpb4[:, :])
                rcpbs[qc] = rcpb

            def emit_bc(qc):
                # broadcast 1/denom rows to all 128 partitions (h0 rows on
                # partitions 0-63, h1 on 64-127) via two K=1 matmuls
                rcpb = rcpbs.pop(qc)
                bcps = ps2.tile([128, 512], F32, tag="s2", name=f"bc{qc}")
                for h in (0, 1):
                    nc.tensor.matmul(
                        bcps[h * 64:(h + 1) * 64, :],
                        lhsT=ones_sb[0:1, 0:64],
                        rhs=rcpb[0:1, h * 512:(h + 1) * 512],
                        start=True, stop=True)
                rb = npool.tile([128, 512], F32, tag="rb")
                nc.vector.tensor_copy(out=rb[:], in_=bcps[:])
                if debug and qc == 0:
                    nc.sync.dma_start(dbg_rb[:, :], rb[0:64, :])
                rbs[qc] = rb

            def emit_av(qc, kt, acc):
                # the two heads run as concurrent PE column-tiles:
                # h0 -> output partitions 0-63, h1 -> 64-127
                pt = pts.pop((qc, kt))
                for h in (0, 1):
                    nc.tensor.matmul(
                        acc[h * 64:(h + 1) * 64, :],
                        lhsT=v_sb[:, kt, h * 64:(h + 1) * 64],
                        rhs=pt[:, h * 512:(h + 1) * 512],
                        start=(kt == 0), stop=(kt == KT - 1))

            def fire_ag(j, js):
                nc.gpsimd.dma_start(bounce[j][:, :], attnt_sb[:, js])
                nc.gpsimd.collective_compute(
                    "AllGather",
                    mybir.AluOpType.bypass,
                    replica_groups=[list(range(N_CORES))],
                    ins=[bounce[j].ap().opt()],
                    outs=[ag_out[j].ap().opt()],
                )

            def emit_norm_and_ag(qc, acc):
                # normalize straight out of PSUM (rb is long ready), ship
                qs = slice(qc * QC, (qc + 1) * QC)
                nc.vector.tensor_tensor(
                    attnt_sb[:, qs], acc[:, :], rbs.pop(qc)[:, :], MULT)
                fire_ag(qc, qs)

            # ---- stage 1: chunked projections ----
            _pc = [0]

            def proj_gen(wsb, dest, j, pool=None, ptag=None):
                # one chunk of one projection; single 1-bank PSUM buf,
                # yields every 2 t-steps so scores/exp interleave finely
                _pc[0] += 1
                pool = pool or ps2
                ps = pool.tile([128, 512], F32, tag=ptag or "s2",
                               name=f"pj{_pc[0]}")
                js = slice(j * 512, (j + 1) * 512)
                for t in range(DT):
                    nc.tensor.matmul(
                        ps[:],
                        lhsT=wsb[:, t, :],
                        rhs=xt_sb[:, t, js],
                        start=(t == 0), stop=(t == DT - 1))
                    if t % 2 == 1 and t < DT - 1:
                        yield
                nc.vector.tensor_copy(out=dest[:, js], in_=ps[:])
                yield

            def proj_pair_gen(wsb, dest, j0, pool=None, ptag=None):
                # two 512-col chunks t-major in one [128,1024] PSUM buf:
                # the per-t LDWEIGHTS is amortized over both matmuls
                _pc[0] += 1
                pool = pool or ps2
                ps = pool.tile([128, 1024], F32, tag=ptag or "s2",
                               name=f"pp{_pc[0]}")
                for t in range(DT):
                    for jj in (0, 1):
                        nc.tensor.matmul(
                            ps[:, jj * 512:(jj + 1) * 512],
                            lhsT=wsb[:, t, :],
                            rhs=xt_sb[:, t,
                                      (j0 + jj) * 512:(j0 + jj + 1) * 512],
                            start=(t == 0), stop=(t == DT - 1))
                    yield
                nc.vector.tensor_copy(
                    out=dest[:, j0 * 512:(j0 + 2) * 512], in_=ps[:])
                yield

            def transpose_gen(jv, pool=None, ptag=None):
                # 4 PE transposes of V into one PSUM tile + one grouped copy
                pool = pool or ps2
                tp = pool.tile([128, 4, 128], BF, tag=ptag or "s2",
                               name=f"tp{jv}")
                for i in range(4):
                    st = 4 * jv + i
                    nc.tensor.transpose(
                        tp[:, i, :], vt_sb[:, st * 128:(st + 1) * 128],
                        ident[:])
                    if i == 1:
                        yield
                nc.vector.tensor_copy(
                    out=v_sb[:, 4 * jv:4 * jv + 4, :], in_=tp[:])
                yield

            def run_gen(g):
                for _ in g:
                    pass

            # prologue: K chunk 0 and Q chunk 0 gate the first scores
            # (different PSUM pools, so Q0's matmuls start right behind
            # K0's without waiting for K0's eviction)
            run_gen(proj_gen(wk_sb, kt_sb, 0))
            run_gen(proj_gen(wq_sb, qt_sb, 0, pac, "acc"))

            # deadline-ordered fillers (consumer emission positions):
            # K_j needed by scores step 4j; V_jv+T_jv by AV step 12+4jv;
            # Q_j by step 32j. Emitted at ~3 yields/step.
            from collections import deque
            fillers = deque()
            fillers.append(proj_pair_gen(wk_sb, kt_sb, 1))
            fillers.append(proj_gen(wv_sb, vt_sb, 0, pac, "acc"))
            fillers.append(proj_gen(wv_sb, vt_sb, 1, pac, "acc"))
            fillers.append(transpose_gen(0, pac, "acc"))
            fillers.append(transpose_gen(1, pac, "acc"))
            fillers.append(proj_pair_gen(wk_sb, kt_sb, 3))
            fillers.append(proj_gen(wv_sb, vt_sb, 2, pac, "acc"))
            fillers.append(proj_gen(wv_sb, vt_sb, 3, pac, "acc"))
            fillers.append(transpose_gen(2, pac, "acc"))
            fillers.append(transpose_gen(3, pac, "acc"))
            fillers.append(proj_pair_gen(wk_sb, kt_sb, 5))
            fillers.append(proj_gen(wv_sb, vt_sb, 4, pac, "acc"))
            fillers.append(transpose_gen(4, pac, "acc"))
            fillers.append(proj_gen(wk_sb, kt_sb, 7, pac, "acc"))
            fillers.append(proj_gen(wv_sb, vt_sb, 5, pac, "acc"))
            fillers.append(transpose_gen(5, pac, "acc"))
            fillers.append(proj_gen(wq_sb, qt_sb, 1, pac, "acc"))
            fillers.append(proj_pair_gen(wv_sb, vt_sb, 6))
            fillers.append(transpose_gen(6, pac, "acc"))
            fillers.append(transpose_gen(7, pac, "acc"))
            fillers.append(proj_pair_gen(wq_sb, qt_sb, 2))
            fillers.append(proj_gen(wq_sb, qt_sb, 4, pac, "acc"))
            fillers.append(proj_gen(wq_sb, qt_sb, 5, pac, "acc"))
            fillers.append(proj_pair_gen(wq_sb, qt_sb, 6))

            def filler_step():
                while fillers:
                    try:
                        next(fillers[0])
                        return
                    except StopIteration:
                        fillers.popleft()

            ag_sb = cpool.tile([128, DT, S], BF, tag="big")

            # ---- stage 2: flat software-pipelined attention loop ----
            ESLAG = 8
            seq = [(qc, kt) for qc in range(NQC) for kt in range(KT)]
            accs = {}
            dd_at = {}
            bc_at = {}

            def do_av(g):
                qc, kt = seq[g]
                if kt == 0:
                    accs[qc] = pac.tile([128, 512], F32, tag="acc",
                                        name=f"acc{qc}")
                emit_av(qc, kt, accs[qc])
                if kt == KT - 1:
                    emit_norm_and_ag(qc, accs.pop(qc))
                    nc.sync.dma_start(
                        ag_sb[:, :, qc * QC:(qc + 1) * QC],
                        ag_out[qc].rearrange("(t p) s -> p t s", p=128))

            av_at = {}
            for g, (qc, kt) in enumerate(seq):
                lag = (LAG if qc < NQC - 2 else
                       (20 if qc == NQC - 2 else 16))
                av_at.setdefault(g + lag, []).append(g)
            for qc in range(NQC):
                g31 = qc * KT + KT - 1
                if qc < NQC - 1:
                    dd_at[g31 + ESLAG + 8] = qc
                    bc_at[g31 + ESLAG + 12] = qc
                else:
                    dd_at[g31 + ESLAG + 1] = qc
                    bc_at[g31 + ESLAG + 3] = qc

            for g in range(len(seq) + LAG):
                if g < len(seq):
                    emit_scores(*seq[g])
                filler_step()
                filler_step()
                if g < 21:
                    filler_step()
                if ESLAG <= g < len(seq) + ESLAG:
                    emit_esum(*seq[g - ESLAG])
                if g in dd_at:
                    emit_dd(dd_at.pop(g))
                if g in bc_at:
                    emit_bc(bc_at.pop(g))
                for gg in av_at.pop(g, ()):
                    do_av(gg)

            # ---- stage 4: output projection (ag_sb chunks already
            # loaded as each AllGather completed) ----
            for j in range(S // 512):
                pool, tag = ((ps2, "s2") if j % 2 == 0 else (pac, "acc"))
                ps = pool.tile([128, 512], F32, tag=tag, name=f"po{j}")
                for t in range(DT):
                    nc.tensor.matmul(
                        ps[:],
                        lhsT=wo_sb[:, t, :],
                        rhs=ag_sb[:, t, j * 512:(j + 1) * 512],
                        start=(t == 0), stop=(t == DT - 1))
                nc.vector.tensor_copy(
                    out=outsb[:, j * 512:(j + 1) * 512], in_=ps[:])
                nc.sync.dma_start(out_d[:, j * 512:(j + 1) * 512],
                                  outsb[:, j * 512:(j + 1) * 512])

    nc.finalize()
    return nc


def _get_nc():
    if "nc" not in _cached:
        _cached["nc"] = _build()
    return _cached["nc"]


def _tile_w(wslice):
    # [1024, 128] -> [128, DT*128] partition-major tiling (bf16, contiguous)
    bf16 = ml_dtypes.bfloat16
    return np.ascontiguousarray(
        wslice.reshape(DT, 128, EC).transpose(1, 0, 2).reshape(128, DT * EC)
    ).astype(bf16)


def _prep_inputs(x, Wq, Wk, Wv, Wo):
    bf16 = ml_dtypes.bfloat16
    x2d = np.asarray(x, dtype=np.float32).reshape(S, DIM)
    xt = np.ascontiguousarray(x2d.T).astype(bf16)
    Wq = np.asarray(Wq, dtype=np.float32)
    Wk = np.asarray(Wk, dtype=np.float32)
    Wv = np.asarray(Wv, dtype=np.float32)
    Wo = np.asarray(Wo, dtype=np.float32)
    in_maps = []
    for c in range(N_CORES):
        sl = slice(c * EC, (c + 1) * EC)
        in_maps.append({
            "xt": xt,
            "wqt": _tile_w(Wq[sl].T),
            "wkt": _tile_w(Wk[sl].T),
            "wvt": _tile_w(Wv[sl].T),
            "wot": _tile_w(Wo[sl].T),
        })
    return in_maps


def run(x, Wq, Wk, Wv, Wo, trace=False):
    """Run the SPMD kernel; returns (out [1,S,DIM] f32, BassKernelResults)."""
    from concourse.bass_utils import run_bass_kernel_spmd

    if trace:
        try:
            import profhook
            profhook.install()
        except Exception:
            pass
    nc = _get_nc()
    in_maps = _prep_inputs(x, Wq, Wk, Wv, Wo)
    res = run_bass_kernel_spmd(nc, in_maps, core_ids=list(range(N_CORES)),
                               trace=trace)
    out = np.empty((1, S, DIM), dtype=np.float32)
    for c in range(N_CORES):
        out[0, :, c * EC:(c + 1) * EC] = res.results[c]["out_t"].T
    return out, res


def kernel(x, mask, Wq, Wk, Wv, Wo):
    # mask is all-zeros by problem spec; it is not applied on device.
    out, _ = run(x, Wq, Wk, Wv, Wo, trace=False)
    return out


# revision 38
# speedup vs baseline: 1.1073x; 1.0020x over previous
"""Multi-head attention (B=1, S=4096, dim=1024, H=16, hd=64) on 8 TRN2 cores.

Sharding: tensor-parallel over heads — 2 heads per core. Wq/Wk/Wv are
column-split (each core computes its 128 output dims of Q/K/V), Wo row-split
(each core computes the full-depth contraction for its 128 output columns
after an AllGather of the per-core attention outputs).

Layout strategy: everything on device is transposed so that every matmul
contraction lands on the partition axis with zero DMA transposes:
  - host passes x.T, pre-tiled Wq.T/Wk.T/Wv.T/Wo.T slices (all bf16)
  - Q.T, K.T, V.T computed as [e, s] (e on partitions); V then PE-transposed
    per 128x128 tile into [s, e]
  - scores computed transposed: S_T[k, q]; the two heads occupy disjoint
    PE row groups (rows 0-63 / 64-127) so their matmuls run concurrently
  - softmax: exp on ScalarE (scale=1/8 folded in, no max subtraction —
    scores are N(0, ~0.41^2), |s|max ~4); ScalarE is the bottleneck engine
    (1 elem/cycle/lane), so everything else is scheduled under its ~1us
    per-(qc,kt) exp cadence
  - AV: the two heads run as concurrent PE column-tiles (h0 -> output
    partitions 0-63, h1 -> 64-127), accumulating a [128, 512] f32 PSUM
    tile per q-chunk (double-buffered)
  - softmax denominators: the otherwise-idle DVE accumulates
    esum[p,q] += exp-tile (bf16) per k-tile; once per q-chunk a tiny
    M=1 ones-matmul reduces esum over partitions, a [128,8]-reshaped
    reciprocal (via 2 tiny DMAs) inverts it, and two K=1 broadcast
    matmuls expand 1/denom to all 128 partitions; the normalize multiply
    reads the AV accumulator straight out of PSUM
  - the ramp: x is DMA'd in 512-column chunks with the K projection
    starting on chunk 0 immediately; remaining K/V/Q projections and V
    transposes are emitted as deadline-ordered fine-grained filler
    generators (t-major pairs amortize LDWEIGHTS) that ping-pong between
    the spare PSUM slot and the (initially free) AV accumulator slots
  - AllGather of normalized attn.T (bf16), one 512-column chunk per
    q-chunk, fired the moment its normalize completes; bounce + norm
    reshape DMAs ride the gpsimd queue, gathered-chunk loads own the
    sync queue (a queued DMA blocks its queue until its input is ready,
    so AG-gated loads must not share a queue with anything earlier)
  - output projection per gathered chunk overlaps the remaining AGs;
    out.T [128 e_out, 4096 s] is transposed on the host.
"""

import numpy as np
import ml_dtypes

N_CORES = 8
S = 4096
DIM = 1024
HD = 64
EC = 128          # output dims (= 2 heads * 64) per core
QC = 512          # q-chunk width in the main loop
NQC = S // QC
KT = S // 128     # 32 k-tiles
DT = DIM // 128   # 8 d-tiles
NAG = 8           # AllGather chunks along s
SAG = S // NAG
LAG = 24          # AV software-pipeline depth (k-tiles behind scores)

_cached = {}


def _build(debug=False):
    import concourse.bass as bass
    import concourse.mybir as mybir
    import concourse.tile as tile
    from concourse import bacc
    from concourse.masks import make_identity

    BF = mybir.dt.bfloat16
    F32 = mybir.dt.float32
    MULT = mybir.AluOpType.mult
    EXP = mybir.ActivationFunctionType.Exp

    nc = bacc.Bacc("TRN2", target_bir_lowering=False, debug=False,
                   num_devices=N_CORES)

    xt_d = nc.declare_dram_parameter("xt", [DIM, S], BF, isOutput=False)
    wqt_d = nc.declare_dram_parameter("wqt", [128, DT * EC], BF, isOutput=False)
    wkt_d = nc.declare_dram_parameter("wkt", [128, DT * EC], BF, isOutput=False)
    wvt_d = nc.declare_dram_parameter("wvt", [128, DT * EC], BF, isOutput=False)
    wot_d = nc.declare_dram_parameter("wot", [128, DT * EC], BF, isOutput=False)
    out_d = nc.declare_dram_parameter("out_t", [EC, S], F32, isOutput=True)

    AGW = [SAG] * NAG
    bounce = [nc.dram_tensor(f"bounce{j}", [EC, w], BF)
              for j, w in enumerate(AGW)]
    ag_out = [nc.dram_tensor(f"ag_out{j}", [DIM, w], BF, addr_space="Shared")
              for j, w in enumerate(AGW)]

    if debug:
        dbg_denom = nc.declare_dram_parameter("dbg_denom", [1, 512], F32,
                                              isOutput=True)
        dbg_rcp = nc.declare_dram_parameter("dbg_rcp", [1, 512], F32,
                                            isOutput=True)
        dbg_rb = nc.declare_dram_parameter("dbg_rb", [64, 512], F32,
                                           isOutput=True)
        dbg_pt = nc.declare_dram_parameter("dbg_pt", [128, 1024], BF,
                                           isOutput=True)

    with tile.TileContext(nc) as tc:
        with (
            tc.tile_pool(name="const", bufs=1) as cpool,
            tc.tile_pool(name="pt", bufs=LAG + 4) as ptp,
            tc.tile_pool(name="norm", bufs=2) as npool,
            tc.tile_pool(name="esum", bufs=2) as esp,
            tc.tile_pool(name="ps_sc", bufs=2, space="PSUM") as psc,
            tc.tile_pool(name="ps_fl", bufs=1, space="PSUM") as ps2,
            tc.tile_pool(name="ps_acc", bufs=2, space="PSUM") as pac,
        ):
            # ---- persistent SBUF tiles ----
            wq_sb = cpool.tile([128, DT, EC], BF, tag="wq")
            wk_sb = cpool.tile([128, DT, EC], BF, tag="wk")
            wv_sb = cpool.tile([128, DT, EC], BF, tag="wv")
            wo_sb = cpool.tile([128, DT, EC], BF, tag="wo")
            xt_sb = cpool.tile([128, DT, S], BF, tag="big")
            qt_sb = cpool.tile([128, S], BF, tag="qt")
            kt_sb = cpool.tile([128, S], BF, tag="kt")
            vt_sb = cpool.tile([128, S], BF, tag="vt")
            v_sb = cpool.tile([128, KT, 128], BF, tag="v")
            attnt_sb = cpool.tile([128, S], BF, tag="attnt")
            ident = cpool.tile([128, 128], BF, tag="ident")
            ones_sb = cpool.tile([128, 64], BF, tag="ones")
            outsb = cpool.tile([128, S], F32, tag="outsb")

            # ---- loads: K weights + x chunk 0 first so the K projection
            # starts immediately; remaining x chunks stream in behind ----
            nc.sync.dma_start(
                wk_sb[:], wkt_d.rearrange("p (o f) -> p o f", o=DT))

            # one trigger per 512-column chunk (3D AP over the 8 row
            # blocks) — the sync sequencer dispatches triggers serially at
            # ~150ns each, so 64 separate transfers would delay the first
            # data (and the whole exp stream) by ~10us
            xt_dv = xt_d.rearrange("(t p) s -> p t s", p=128)

            def load_chunk(j):
                js = slice(j * QC, (j + 1) * QC)
                nc.sync.dma_start(xt_sb[:, :, js], xt_dv[:, :, js])

            load_chunk(0)
            for wsb, wd in ((wq_sb, wqt_d), (wv_sb, wvt_d), (wo_sb, wot_d)):
                nc.sync.dma_start(
                    wsb[:], wd.rearrange("p (o f) -> p o f", o=DT))
            for j in range(1, DT):
                load_chunk(j)
            make_identity(nc, ident[:])
            nc.vector.memset(ones_sb[:], 1.0)

            # load the exp table set (~2.7us) during the x DMA, and keep
            # the PE busy ~3.5us right before the K projection so HAM has
            # unthrottled to full clock when real work starts (gated on
            # ident so it runs at ~10.5us, ending as chunk-0 data lands)
            wrm = ps2.tile([64, 64], F32, tag="s2", name="warm")
            nc.scalar.activation(wrm[0:1, 0:16], ident[0:1, 0:16], EXP)
            for _ in range(40):
                nc.tensor.matmul(wrm[:], lhsT=ident[0:64, 0:64],
                                 rhs=ident[0:64, 64:128], start=True,
                                 stop=True)

            # ---- emission helpers ----
            pts = {}
            esums = {}
            rbs = {}
            rcpbs = {}

            def emit_scores(qc, kt):
                qs = slice(qc * QC, (qc + 1) * QC)
                sc = psc.tile([128, 1024], F32, tag="sc")
                for h in (0, 1):
                    nc.tensor.matmul(
                        sc[:, h * 512:(h + 1) * 512],
                        lhsT=kt_sb[h * 64:(h + 1) * 64,
                                   kt * 128:(kt + 1) * 128],
                        rhs=qt_sb[h * 64:(h + 1) * 64, qs],
                        start=True, stop=True)
                pt = ptp.tile([128, 1024], BF, tag="pt")
                nc.scalar.activation(pt[:], sc[:], EXP, scale=0.125)
                pts[(qc, kt)] = pt
                if debug and qc == 0 and kt == 0:
                    nc.sync.dma_start(dbg_pt[:, :], pt[:])

            def emit_esum(qc, kt):
                # running softmax-denominator accumulation on the (idle)
                # DVE: esum[p, q] = sum_kt pt[p, q]; the partition-axis
                # reduction happens later in one tiny ones-matmul per qc.
                # bf16 accumulation noise on the final denominator is
                # ~0.1% (128 independent partials averaged by the matmul).
                pt = pts[(qc, kt)]
                if kt == 0:
                    es = esp.tile([128, 1024], BF, tag="es", name=f"es{qc}")
                    esums[qc] = es
                    nc.vector.tensor_copy(out=es[:], in_=pt[:])
                else:
                    es = esums[qc]
                    nc.vector.tensor_tensor(es[:], es[:], pt[:],
                                            mybir.AluOpType.add)

            def emit_dd(qc):
                # partition-reduce esum -> per-q denominators [1, 1024],
                # then 128-lane reciprocal via tiny reshape DMAs
                es = esums.pop(qc)
                dd = ps2.tile([1, 1024], F32, tag="s2", name=f"dd{qc}")
                for h in (0, 1):
                    nc.tensor.matmul(
                        dd[0:1, h * 512:(h + 1) * 512],
                        lhsT=ones_sb[:, 0:1],
                        rhs=es[:, h * 512:(h + 1) * 512],
                        start=True, stop=True)
                dd_sb = npool.tile([1, 1024], F32, tag="dd")
                nc.vector.tensor_copy(out=dd_sb[0:1, :], in_=dd[0:1, :])
                den = npool.tile([128, 8], F32, tag="den")
                nc.gpsimd.dma_start(den[:, :], dd_sb[0:1, :])
                rcp = npool.tile([128, 8], F32, tag="rcp")
                nc.vector.reciprocal(rcp[:, :], den[:, :])
                rcpb4 = npool.tile([128, 8], BF, tag="rcpb4")
                nc.vector.tensor_copy(out=rcpb4[:], in_=rcp[:])
                rcpb = npool.tile([1, 1024], BF, tag="rcpb")
                nc.gpsimd.dma_start(rcpb[0:1, :], rcpb4[:, :])
                rcpbs[qc] = rcpb

            def emit_bc(qc):
                # broadcast 1/denom rows to all 128 partitions (h0 rows on
                # partitions 0-63, h1 on 64-127) via two K=1 matmuls
                rcpb = rcpbs.pop(qc)
                bcps = ps2.tile([128, 512], F32, tag="s2", name=f"bc{qc}")
                for h in (0, 1):
                    nc.tensor.matmul(
                        bcps[h * 64:(h + 1) * 64, :],
                        lhsT=ones_sb[0:1, 0:64],
                        rhs=rcpb[0:1, h * 512:(h + 1) * 512],
                        start=True, stop=True)
                rb = npool.tile([128, 512], F32, tag="rb")
                nc.vector.tensor_copy(out=rb[:], in_=bcps[:])
                if debug and qc == 0:
                    nc.sync.dma_start(dbg_rb[:, :], rb[0:64, :])
                rbs[qc] = rb

            def emit_av(qc, kt, acc):
                # the two heads run as concurrent PE column-tiles:
                # h0 -> output partitions 0-63, h1 -> 64-127
                pt = pts.pop((qc, kt))
                for h in (0, 1):
                    nc.tensor.matmul(
                        acc[h * 64:(h + 1) * 64, :],
                        lhsT=v_sb[:, kt, h * 64:(h + 1) * 64],
                        rhs=pt[:, h * 512:(h + 1) * 512],
                        start=(kt == 0), stop=(kt == KT - 1))

            def fire_ag(j, js):
                nc.gpsimd.dma_start(bounce[j][:, :], attnt_sb[:, js])
                nc.gpsimd.collective_compute(
                    "AllGather",
                    mybir.AluOpType.bypass,
                    replica_groups=[list(range(N_CORES))],
                    ins=[bounce[j].ap().opt()],
                    outs=[ag_out[j].ap().opt()],
                )

            def emit_norm_and_ag(qc, acc):
                # normalize straight out of PSUM (rb is long ready), ship
                qs = slice(qc * QC, (qc + 1) * QC)
                nc.vector.tensor_tensor(
                    attnt_sb[:, qs], acc[:, :], rbs.pop(qc)[:, :], MULT)
                fire_ag(qc, qs)

            # ---- stage 1: chunked projections ----
            _pc = [0]

            def proj_gen(wsb, dest, j, pool=None, ptag=None):
                # one chunk of one projection; single 1-bank PSUM buf,
                # yields every 2 t-steps so scores/exp interleave finely
                _pc[0] += 1
                pool = pool or ps2
                ps = pool.tile([128, 512], F32, tag=ptag or "s2",
                               name=f"pj{_pc[0]}")
                js = slice(j * 512, (j + 1) * 512)
                for t in range(DT):
                    nc.tensor.matmul(
                        ps[:],
                        lhsT=wsb[:, t, :],
                        rhs=xt_sb[:, t, js],
                        start=(t == 0), stop=(t == DT - 1))
                    if t % 2 == 1 and t < DT - 1:
                        yield
                nc.vector.tensor_copy(out=dest[:, js], in_=ps[:])
                yield

            def proj_pair_gen(wsb, dest, j0, pool=None, ptag=None):
                # two 512-col chunks t-major in one [128,1024] PSUM buf:
                # the per-t LDWEIGHTS is amortized over both matmuls
                _pc[0] += 1
                pool = pool or ps2
                ps = pool.tile([128, 1024], F32, tag=ptag or "s2",
                               name=f"pp{_pc[0]}")
                for t in range(DT):
                    for jj in (0, 1):
                        nc.tensor.matmul(
                            ps[:, jj * 512:(jj + 1) * 512],
                            lhsT=wsb[:, t, :],
                            rhs=xt_sb[:, t,
                                      (j0 + jj) * 512:(j0 + jj + 1) * 512],
                            start=(t == 0), stop=(t == DT - 1))
                    yield
                nc.vector.tensor_copy(
                    out=dest[:, j0 * 512:(j0 + 2) * 512], in_=ps[:])
                yield

            def transpose_gen(jv, pool=None, ptag=None):
                # 4 PE transposes of V into one PSUM tile + one grouped copy
                pool = pool or ps2
                tp = pool.tile([128, 4, 128], BF, tag=ptag or "s2",
                               name=f"tp{jv}")
                for i in range(4):
                    st = 4 * jv + i
                    nc.tensor.transpose(
                        tp[:, i, :], vt_sb[:, st * 128:(st + 1) * 128],
                        ident[:])
                    if i == 1:
                        yield
                nc.vector.tensor_copy(
                    out=v_sb[:, 4 * jv:4 * jv + 4, :], in_=tp[:])
                yield

            def run_gen(g):
                for _ in g:
                    pass

            # prologue: K chunk 0 and Q chunk 0 gate the first scores
            # (different PSUM pools, so Q0's matmuls start right behind
            # K0's without waiting for K0's eviction)
            run_gen(proj_gen(wk_sb, kt_sb, 0))
            run_gen(proj_gen(wq_sb, qt_sb, 0, pac, "acc"))

            # deadline-ordered fillers (consumer emission positions):
            # K_j needed by scores step 4j; V_jv+T_jv by AV step 12+4jv;
            # Q_j by step 32j. Emitted at ~3 yields/step.
            from collections import deque
            fillers = deque()
            fillers.append(proj_pair_gen(wk_sb, kt_sb, 1))
            fillers.append(proj_gen(wv_sb, vt_sb, 0, pac, "acc"))
            fillers.append(proj_gen(wv_sb, vt_sb, 1, pac, "acc"))
            fillers.append(transpose_gen(0, pac, "acc"))
            fillers.append(transpose_gen(1, pac, "acc"))
            fillers.append(proj_pair_gen(wk_sb, kt_sb, 3))
            fillers.append(proj_gen(wv_sb, vt_sb, 2, pac, "acc"))
            fillers.append(proj_gen(wv_sb, vt_sb, 3, pac, "acc"))
            fillers.append(transpose_gen(2, pac, "acc"))
            fillers.append(transpose_gen(3, pac, "acc"))
            fillers.append(proj_pair_gen(wk_sb, kt_sb, 5))
            fillers.append(proj_gen(wv_sb, vt_sb, 4, pac, "acc"))
            fillers.append(transpose_gen(4, pac, "acc"))
            fillers.append(proj_gen(wk_sb, kt_sb, 7, pac, "acc"))
            fillers.append(proj_gen(wv_sb, vt_sb, 5, pac, "acc"))
            fillers.append(transpose_gen(5, pac, "acc"))
            fillers.append(proj_gen(wq_sb, qt_sb, 1, pac, "acc"))
            fillers.append(proj_pair_gen(wv_sb, vt_sb, 6))
            fillers.append(transpose_gen(6, pac, "acc"))
            fillers.append(transpose_gen(7, pac, "acc"))
            fillers.append(proj_pair_gen(wq_sb, qt_sb, 2))
            fillers.append(proj_gen(wq_sb, qt_sb, 4, pac, "acc"))
            fillers.append(proj_gen(wq_sb, qt_sb, 5, pac, "acc"))
            fillers.append(proj_pair_gen(wq_sb, qt_sb, 6))

            def filler_step():
                while fillers:
                    try:
                        next(fillers[0])
                        return
                    except StopIteration:
                        fillers.popleft()

            ag_sb = cpool.tile([128, DT, S], BF, tag="big")

            # ---- stage 2: flat software-pipelined attention loop ----
            ESLAG = 8
            seq = [(qc, kt) for qc in range(NQC) for kt in range(KT)]
            accs = {}
            dd_at = {}
            bc_at = {}

            def do_av(g):
                qc, kt = seq[g]
                if kt == 0:
                    accs[qc] = pac.tile([128, 512], F32, tag="acc",
                                        name=f"acc{qc}")
                emit_av(qc, kt, accs[qc])
                if kt == KT - 1:
                    emit_norm_and_ag(qc, accs.pop(qc))
                    # per-t loads (not coalesced): the output projection's
                    # t-matmuls pipeline against the progressive row-block
                    # arrivals, which matters for the tail-critical chunk
                    for t in range(DT):
                        nc.sync.dma_start(
                            ag_sb[:, t, slice(qc * QC, (qc + 1) * QC)],
                            ag_out[qc][t * 128:(t + 1) * 128, :])

            av_at = {}
            for g, (qc, kt) in enumerate(seq):
                lag = (LAG if qc < NQC - 2 else
                       (20 if qc == NQC - 2 else 16))
                av_at.setdefault(g + lag, []).append(g)
            for qc in range(NQC):
                g31 = qc * KT + KT - 1
                if qc < NQC - 1:
                    dd_at[g31 + ESLAG + 8] = qc
                    bc_at[g31 + ESLAG + 12] = qc
                else:
                    dd_at[g31 + ESLAG + 1] = qc
                    bc_at[g31 + ESLAG + 3] = qc

            for g in range(len(seq) + LAG):
                if g < len(seq):
                    emit_scores(*seq[g])
                filler_step()
                filler_step()
                if g < 21:
                    filler_step()
                if ESLAG <= g < len(seq) + ESLAG:
                    emit_esum(*seq[g - ESLAG])
                if g in dd_at:
                    emit_dd(dd_at.pop(g))
                if g in bc_at:
                    emit_bc(bc_at.pop(g))
                for gg in av_at.pop(g, ()):
                    do_av(gg)

            # ---- stage 4: output projection (ag_sb chunks already
            # loaded as each AllGather completed) ----
            for j in range(S // 512):
                pool, tag = ((ps2, "s2") if j % 2 == 0 else (pac, "acc"))
                ps = pool.tile([128, 512], F32, tag=tag, name=f"po{j}")
                for t in range(DT):
                    nc.tensor.matmul(
                        ps[:],
                        lhsT=wo_sb[:, t, :],
                        rhs=ag_sb[:, t, j * 512:(j + 1) * 512],
                        start=(t == 0), stop=(t == DT - 1))
                nc.vector.tensor_copy(
                    out=outsb[:, j * 512:(j + 1) * 512], in_=ps[:])
                nc.sync.dma_start(out_d[:, j * 512:(j + 1) * 512],
                                  outsb[:, j * 512:(j + 1) * 512])

    nc.finalize()
    return nc


def _get_nc():
    if "nc" not in _cached:
        _cached["nc"] = _build()
    return _cached["nc"]


def _tile_w(wslice):
    # [1024, 128] -> [128, DT*128] partition-major tiling (bf16, contiguous)
    bf16 = ml_dtypes.bfloat16
    return np.ascontiguousarray(
        wslice.reshape(DT, 128, EC).transpose(1, 0, 2).reshape(128, DT * EC)
    ).astype(bf16)


def _prep_inputs(x, Wq, Wk, Wv, Wo):
    bf16 = ml_dtypes.bfloat16
    x2d = np.asarray(x, dtype=np.float32).reshape(S, DIM)
    xt = np.ascontiguousarray(x2d.T).astype(bf16)
    Wq = np.asarray(Wq, dtype=np.float32)
    Wk = np.asarray(Wk, dtype=np.float32)
    Wv = np.asarray(Wv, dtype=np.float32)
    Wo = np.asarray(Wo, dtype=np.float32)
    in_maps = []
    for c in range(N_CORES):
        sl = slice(c * EC, (c + 1) * EC)
        in_maps.append({
            "xt": xt,
            "wqt": _tile_w(Wq[sl].T),
            "wkt": _tile_w(Wk[sl].T),
            "wvt": _tile_w(Wv[sl].T),
            "wot": _tile_w(Wo[sl].T),
        })
    return in_maps


def run(x, Wq, Wk, Wv, Wo, trace=False):
    """Run the SPMD kernel; returns (out [1,S,DIM] f32, BassKernelResults)."""
    from concourse.bass_utils import run_bass_kernel_spmd

    if trace:
        try:
            import profhook
            profhook.install()
        except Exception:
            pass
    nc = _get_nc()
    in_maps = _prep_inputs(x, Wq, Wk, Wv, Wo)
    res = run_bass_kernel_spmd(nc, in_maps, core_ids=list(range(N_CORES)),
                               trace=trace)
    out = np.empty((1, S, DIM), dtype=np.float32)
    for c in range(N_CORES):
        out[0, :, c * EC:(c + 1) * EC] = res.results[c]["out_t"].T
    return out, res


def kernel(x, mask, Wq, Wk, Wv, Wo):
    # mask is all-zeros by problem spec; it is not applied on device.
    out, _ = run(x, Wq, Wk, Wv, Wo, trace=False)
    return out
